# revision 6
# baseline (speedup 1.0000x reference)
"""NT-Xent / SimCLR contrastive loss on 8 Trainium2 NeuronCores, v2.

Exploits gram symmetry: each unordered 128x128 tile pair of the 64x64
tile grid is exp'd ONCE (2080 tiles vs 4096), halving ScalarE exp work.
Core c (input rolled by -128c samples) computes local row-tiles
L in {0,8,...,56} against col-tiles (L+d) mod 64, d=0..31, plus d=32
tiles for L in {0,8,16,24}. Row sums via exp accum_out; transpose-side
column sums accumulate tile-wise into a [128, 8192] bf16 SBUF
accumulator (DVE adds; fully-virgin chunks of rows 0/32 write exp
output straight into the accumulator). Per 512-col region, once its
last covering row-tile is done, a ones-stationary matmul reduces the
128 partitions into a dedicated PSUM bank (4 regions per bank at
partition offsets 0/32/64/96) and a grouped DVE copy moves 4 regions
at once to SBUF for DMA-out. Positives are computed exactly as
elementwise zn-block products (DVE) summed per-feature; host sums the
rest. Host combines partial row/col sums across cores, subtracts the
self-sim e^2, and takes log — the cross-core reduction the sharding
hint assigns to the final gather.

Chunk layout per row-tile: A = d-cols 128..1663, B = 1664..3199,
C = 3200..4095 + the deferred d0 block (+ d32 block). The d0 block
never enters the accumulator (its transpose half is its own row sum).
"""

import numpy as np

P = 128
NS = 8192            # 2N
D = 128
NCORES = 8
NT = NS // P         # 64 col tiles
TEMP = 0.5
INV_T = 1.0 / TEMP
ORDER = [0, 8, 16, 24, 32, 40, 48, 56]  # processing order of row-tiles
D32ROWS = (0, 8, 16, 24)
NCH = 4              # norm chunks
CHW = NS // NCH      # 2048
CT = CHW // P        # 16
PSW = 1536           # gram PSUM chunk width (3 banks)
NREG = 16
REGW = NS // NREG

_CACHE = {}


def _ensure_paths():
    import sys
    for p in ("/root/.axon_site", "/root/.axon_site/_ro/trn_rl_repo",
              "/root/.axon_site/_ro/pypackages", "/opt/trn_rl_repo", "/opt/pypackages"):
        if p not in sys.path:
            sys.path.append(p)


def _mm_segs(L, pieces):
    """Split k-pieces into <=512-wide matmul segments that never cross
    the mod-8192 wrap. pieces: list of (k0, k1) over the row's d-cols,
    or ("abs", col, w) for absolute column blocks."""
    segs = []
    off = 0
    for pc in pieces:
        if pc[0] == "abs":
            _, c0, w = pc
            segs.append((off, c0, w))
            off += w
            continue
        k0, k1 = pc
        k = k0
        while k < k1:
            c = (128 * L + k) % NS
            # cap at the PSUM bank boundary (512 f32) and the mod-NS wrap
            w = min(512 - (off % 512), k1 - k, NS - c)
            segs.append((off, c, w))
            off += w
            k += w
    return segs, off


def _plan():
    """Static schedule. Returns list of steps, each:
    L, width, segs [(psum_off, col, w)], target ("acc"|"esc"),
    adds [(esc_off, dst_col, w, is_copy)], regions [r...]."""
    steps = []
    written = np.zeros(NS, dtype=bool)
    solo = []
    for L in ORDER:
        chunks = [[(128, 1664)], [(1664, 3200)]]
        cpieces = [(3200, 4096), (0, 128)]          # k-tail + deferred d0
        chunks.append(cpieces)
        if L == 24:
            # row 0's d32 tile, deferred here so it never waits on zn ch2
            chunks.append([("abs", 32 * 128, 128)])
        for ci, pieces in enumerate(chunks):
            is_c = ci == 2
            if ci == 3:
                is_c = False
            if is_c and L in D32ROWS and L != 0:
                pieces = pieces + [("abs", (L + 32) * 128, 128)]
            segs, width = _mm_segs(L, pieces)
            # acc-target cols for this chunk: all except the d0 piece
            acc_runs = []   # (esc_off, dst_col, w) contiguous in dst
            off = 0
            for pc in pieces:
                if pc[0] == "abs":
                    acc_runs.append((off, pc[1], pc[2]))
                    off += pc[2]
                    continue
                k0, k1 = pc
                if (k0, k1) == (0, 128):   # d0: skip acc entirely
                    off += 128
                    continue
                k = k0
                while k < k1:
                    c = (128 * L + k) % NS
                    w = min(k1 - k, NS - c)
                    acc_runs.append((off, c, w))
                    off += w
                    k += w
            virgin = all(not written[c:c + w].any() for _, c, w in acc_runs)
            direct = virgin and not is_c and len(acc_runs) == 1
            adds = []
            if not direct:
                for eoff, c, w in acc_runs:
                    i = 0
                    while i < w:
                        v = bool(written[c + i])
                        j = i
                        while j < w and bool(written[c + j]) == v:
                            j += 1
                        adds.append((eoff + i, c + i, j - i, not v))
                        i = j
            for _, c, w in acc_runs:
                written[c:c + w] = True
            steps.append(dict(L=0 if ci == 3 else L, width=width, segs=segs,
                              target="acc" if direct else "esc",
                              acc_col=acc_runs[0][1] if direct else None,
                              adds=adds))
    assert written.all()
    reg_last = [-1] * NREG
    for si, st in enumerate(steps):
        wr = ([(st["acc_col"], st["width"])] if st["target"] == "acc"
              else [(c, w) for _, c, w, _ in st["adds"]])
        for c, w in wr:
            for r in range(c // REGW, (c + w - 1) // REGW + 1):
                reg_last[r] = si
    order = []
    for si, st in enumerate(steps):
        st["regions"] = [r for r in range(NREG) if reg_last[r] == si]
        order.extend(st["regions"])
    assert len(order) == NREG
    return steps


def _build():
    _ensure_paths()
    import concourse.bass as bass
    import concourse.bacc as bacc
    import concourse.mybir as mybir
    import concourse.tile as tile

    dt_bf = mybir.dt.bfloat16
    dt_f32 = mybir.dt.float32
    AFT = mybir.ActivationFunctionType
    AX = mybir.AxisListType
    ALU = mybir.AluOpType

    steps = _plan()

    nc = bacc.Bacc("TRN2", target_bir_lowering=False, debug=False,
                   num_devices=NCORES)

    znat_d = nc.dram_tensor("znat", [P, NT, P], dt_bf, kind="ExternalInput")
    zt_d = nc.dram_tensor("zt", [P, NS], dt_bf, kind="ExternalInput")
    rs_d = nc.dram_tensor("out_rs", [P, 25], dt_f32, kind="ExternalOutput")
    cs_d = nc.dram_tensor("out_cs", [4, 4, REGW], dt_f32, kind="ExternalOutput")
    pos_d = nc.dram_tensor("out_pos", [P, 4], dt_f32, kind="ExternalOutput")
    rrow_dram = nc.dram_tensor("rrow_scratch", [1, NS], dt_bf)

    with tile.TileContext(nc) as tc:
        with (
            tc.tile_pool(name="big", bufs=1) as big,
            tc.tile_pool(name="work", bufs=2) as work,
            tc.tile_pool(name="psum", bufs=2, space=bass.MemorySpace.PSUM) as psum,
        ):
            zt = big.tile([P, NS], dt_bf, tag="zt")
            zna = big.tile([P, NT, P], dt_bf, tag="zna")
            zn = big.tile([P, NS], dt_bf, tag="zn")
            rb = big.tile([P, NS], dt_bf, tag="rb")
            acc = big.tile([P, NS], dt_bf, tag="acc")
            ss = big.tile([P, NT], dt_f32, tag="ss")
            sn = big.tile([P, NT], dt_f32, tag="sn")
            rf = big.tile([P, NT], dt_f32, tag="rf")
            rs = big.tile([P, 25], dt_f32, tag="rs")
            posb = big.tile([P, 4], dt_f32, tag="posb")
            ones = big.tile([P, 32], dt_bf, tag="ones")
            cs_sb = big.tile([P, 2 * REGW], dt_f32, tag="cs_sb")
            nvb = [big.tile([P, P], dt_bf, tag=f"nvb{i}", name=f"nvb{i}")
                   for i in range(2)]
            scr = big.tile([P, 1], dt_f32, tag="scr")
            scr2 = big.tile([P, 1], dt_f32, tag="scr2")

            # sqrt-table preload under the input DMAs
            nc.gpsimd.memset(scr[:], 1.0)
            nc.scalar.activation(scr2[:], scr[:], AFT.Sqrt)
            nc.gpsimd.memset(ones[:], 1.0)
            for t in nvb:
                nc.gpsimd.memset(t[:], 1.0)

            for h in range(NCH):
                nc.sync.dma_start(zna[:, h * CT:(h + 1) * CT, :],
                                  znat_d[:, h * CT:(h + 1) * CT, :])
            for h in range(2):
                nc.sync.dma_start(zt[:, h * NS // 2:(h + 1) * NS // 2],
                                  zt_d[:, h * NS // 2:(h + 1) * NS // 2])

            # ---- norms ----
            for h in range(NCH):
                tsl = slice(h * CT, (h + 1) * CT)
                csl = slice(h * CHW, (h + 1) * CHW)
                prio = tc.high_priority() if h == 0 else None
                if prio is not None:
                    prio.__enter__()
                sq = work.tile([P, CT, P], dt_bf, tag="sq")
                nc.vector.tensor_tensor(sq[:], zna[:, tsl, :], zna[:, tsl, :],
                                        ALU.mult)
                sqh = work.tile([P, CT, P // 2], dt_bf, tag="sqh")
                nc.vector.tensor_tensor(sqh[:], sq[:, :, 0:64],
                                        sq[:, :, 64:128], ALU.add)
                nc.vector.reduce_sum(ss[:, tsl], sqh[:], axis=AX.X)
                nc.scalar.activation(sn[:, tsl], ss[:, tsl], AFT.Sqrt)
                nv = nvb[h % 2]
                with nc.allow_low_precision("bf16 zn path tolerates it"):
                    nc.vector.reciprocal(nv[:, 0:CT], sn[:, tsl])
                nvt = work.tile([P, P], dt_bf, tag="nvt", bufs=4)
                nc.scalar.dma_start_transpose(nvt[:], nv[:])
                nc.scalar.dma_start(rrow_dram[0:1, csl], nvt[0:CT, :])
                nc.sync.dma_start(rb[:, csl],
                                  rrow_dram[0:1, csl].broadcast_to([P, CHW]))
                if prio is not None:
                    prio.__exit__(None, None, None)
            # first chunk's multiply split in half: the first gram
            # matmuls need only cols < 1152 and start one DVE op earlier
            nc.vector.tensor_tensor(zn[:, 0:1024], zt[:, 0:1024],
                                    rb[:, 0:1024], ALU.mult)
            nc.vector.tensor_tensor(zn[:, 1024:CHW], zt[:, 1024:CHW],
                                    rb[:, 1024:CHW], ALU.mult)
            for h in range(1, NCH):
                csl = slice(h * CHW, (h + 1) * CHW)
                nc.vector.tensor_tensor(zn[:, csl], zt[:, csl], rb[:, csl],
                                        ALU.mult)

            # exp-table preload; input depends on the last sqrt so the
            # scheduler cannot hoist it before the sqrts (table thrash)
            nc.scalar.activation(scr2[:], sn[:, NT - 1:NT], AFT.Exp)

            # ---- gram / exp / colsum ----
            d32k = 0
            csg = 0          # colsum group counter (4 regions per group)
            cs_meta = []     # (region, group, slot)
            cs_ps = None
            for rcol, st in enumerate(steps):
                L = st["L"]
                stat = zn[:, 128 * L:128 * L + 128]
                W = st["width"]
                ps = psum.tile([P, PSW], dt_f32, tag="ps")
                for off, col, w in st["segs"]:
                    nc.tensor.matmul(ps[:, off:off + w], stat,
                                     zn[:, col:col + w],
                                     start=True, stop=True)
                if st["target"] == "acc":
                    c0 = st["acc_col"]
                    nc.scalar.activation(acc[:, c0:c0 + W], ps[:, 0:W],
                                         AFT.Exp, scale=INV_T,
                                         accum_out=rs[:, rcol:rcol + 1])
                else:
                    esc = work.tile([P, PSW], dt_bf, tag="esc", bufs=4)
                    nc.scalar.activation(esc[:, 0:W], ps[:, 0:W], AFT.Exp,
                                         scale=INV_T,
                                         accum_out=rs[:, rcol:rcol + 1])
                    for eoff, c, w, is_copy in st["adds"]:
                        if is_copy:
                            nc.vector.tensor_copy(acc[:, c:c + w],
                                                  esc[:, eoff:eoff + w])
                        else:
                            nc.vector.tensor_tensor(acc[:, c:c + w],
                                                    acc[:, c:c + w],
                                                    esc[:, eoff:eoff + w],
                                                    ALU.add)
                # positives for the d32 rows, from zn directly
                if st["L"] in D32ROWS and st["segs"][-1][1] == (L + 32) * 128:
                    a, b = 128 * L, (L + 32) * 128
                    ppd = work.tile([P, P], dt_bf, tag="ppd")
                    nc.vector.tensor_tensor(ppd[:], zn[:, a:a + 128],
                                            zn[:, b:b + 128], ALU.mult)
                    nc.vector.reduce_sum(posb[:, d32k:d32k + 1], ppd[:],
                                         axis=AX.X)
                    d32k += 1
                # colsum region reduction via ones-matmul
                for r in st["regions"]:
                    slot = len(cs_meta) % 4
                    if slot == 0:
                        cs_ps = psum.tile([P, REGW], dt_f32, tag="csps",
                                          name="cs_ps")
                    nc.tensor.matmul(cs_ps[32 * slot:32 * slot + 32, :],
                                     ones[:], acc[:, r * REGW:(r + 1) * REGW],
                                     start=True, stop=True,
                                     tile_position=(0, 32 * slot))
                    cs_meta.append((r, csg, slot))
                    if slot == 3:
                        gcol = (csg % 2) * REGW
                        nc.vector.tensor_copy(
                            cs_sb[:, gcol:gcol + REGW], cs_ps[:])
                        nc.sync.dma_start(
                            cs_d[csg, :, :],
                            cs_sb[0:97:32, gcol:gcol + REGW])
                        csg += 1

            nc.sync.dma_start(rs_d[:], rs[:])
            nc.sync.dma_start(pos_d[:], posb[:])

    nc.compile()
    return nc


def get_nc():
    if "nc" not in _CACHE:
        _CACHE["nc"] = _build()
    return _CACHE["nc"]


def make_in_maps(proj_1: np.ndarray, proj_2: np.ndarray):
    import ml_dtypes
    z = np.concatenate([np.asarray(proj_1), np.asarray(proj_2)], axis=0)
    zb = z.astype(ml_dtypes.bfloat16)
    in_maps = []
    for c in range(NCORES):
        zc = np.roll(zb, -P * c, axis=0)
        znat = np.ascontiguousarray(zc.reshape(NT, P, P).transpose(1, 0, 2))
        ztr = np.ascontiguousarray(zc.T)
        in_maps.append({"znat": znat, "zt": ztr})
    return in_maps


def finish(results) -> np.ndarray:
    steps = _plan()
    reg_order = [r for st in steps for r in st["regions"]]
    denom = np.zeros(NS, dtype=np.float64)
    pos = 0.0
    for c, r in enumerate(results):
        rsv = np.asarray(r["out_rs"], dtype=np.float64)
        csg = np.asarray(r["out_cs"], dtype=np.float64).reshape(NREG, REGW)
        local = np.zeros(NS)
        for i, reg in enumerate(reg_order):
            local[reg * REGW:(reg + 1) * REGW] = csg[i]
        for rcol, st in enumerate(steps):
            L = st["L"]
            local[128 * L:128 * L + 128] += rsv[:, rcol]
        denom += np.roll(local, P * c)
        pos += float(np.asarray(r["out_pos"], dtype=np.float64).sum())
    denom -= np.exp(2.0)
    loss = (np.log(denom).sum() - 2.0 * INV_T * pos) / float(NS)
    return np.float32(loss)


def kernel(proj_1: np.ndarray, proj_2: np.ndarray) -> np.ndarray:
    _ensure_paths()
    from concourse.bass_utils import run_bass_kernel_spmd
    nc = get_nc()
    in_maps = make_in_maps(proj_1, proj_2)
    res = run_bass_kernel_spmd(nc, in_maps, core_ids=list(range(NCORES)))
    return finish(res.results)
